# revision 1
# baseline (speedup 1.0000x reference)
"""Trainium2 Bass kernel for relative-position attention (nn_AttentionMechanism).

Math (per batch b):
  q,k,v = h@Wq, h@Wk, h@Wv  (biases are zero in this problem)
  scores[l,r] = (q[l].k[r] + q[l].E[l-r+1023] + k[r].E[l-r+1023]) / sqrt(64)
  out = softmax(scores) @ v @ Wd

Sharding: 8 cores = (batch b in 0..3) x (query half lh in 0..1).
Each core computes out rows [lh*512, lh*512+512) for batch b.

Per-core algorithm (T orientation: score tiles are [r partitions, l free]):
  - xT via PE transposes; qT/kT = W^T @ xT matmuls (scaled by 8^-1/4... see SCALE);
    v natural with a 64-wide ones block appended (gives softmax denominators for
    free as extra rows of the PV matmul output).
  - Relative-position terms need a diagonal "shear" gather E[l-r+1023], which no
    TRN2 engine can do on-chip (all gathers share indices per 16-partition group).
    Mechanism: music-transformer stride trick through DRAM:
      kd[r,j] = k[r].E_win[j] (fp16) written with row stride 640, read back
        with row stride 639 -> the read IS rel_k^T (plain HWDGE DMA).
      qd[l,u] = q[l].E_win_rev[u] (fp16) written with row stride 1536, read back
        with row stride 1535 through the HWDGE xbar transpose-DMA -> rel_q^T.
      rel_q+rel_k summed on GPSIMD (idle engine), then one DVE add from the
        content-score PSUM, exp on ScalarE.
  - exp on ScalarE (no max subtraction needed: |scores| <~ 1.5 by construction),
    PV + denominators on PE, per-head normalize, then out-projection.
"""

import sys

sys.path.insert(0, "/opt/trn_rl_repo")

import numpy as np

import concourse.bass as bass
import concourse.mybir as mybir
import concourse.tile as tile
from concourse import bacc
from concourse.bass_utils import run_bass_kernel_spmd

FP32 = mybir.dt.float32
FP16 = mybir.dt.float16
ADD = mybir.AluOpType.add
MULT = mybir.AluOpType.mult
EXP = mybir.ActivationFunctionType.Exp

N_CORES = 8
D, H, HD = 768, 12, 64
LQ, LK = 512, 1024
EW = 1536          # E window rows per core (= LQ + LK + pad)
KD_W = 640         # kd chunk width (639 used + 1 pad col)
QD_W = 1536        # qd row stride
SCALE = 0.35355339059327373  # 8**-0.5 applied to q,k AND E => all terms get /8


def _strided_view(ap, dims, extra_offset):
    """Return a copy of `ap` with its [step,count] pairs and offset replaced."""
    v = ap.copy()
    a = v.ap
    assert len(a) == len(dims), (a, dims)
    for i, d in enumerate(dims):
        a[i] = d
    v.ap = a
    v.offset = v.offset + extra_offset
    return v


def build_nc(repeats=1):
    nc = bacc.Bacc("TRN2", target_bir_lowering=False, debug=False,
                   num_devices=N_CORES)

    hq = nc.dram_tensor("hidden_q_T", [D, LQ], FP32, kind="ExternalInput").ap()
    hkv = nc.dram_tensor("hidden_kv_T", [D, LK], FP32, kind="ExternalInput").ap()
    wq = nc.dram_tensor("Wq", [D, D], FP32, kind="ExternalInput").ap()
    wk = nc.dram_tensor("Wk", [D, D], FP32, kind="ExternalInput").ap()
    wv = nc.dram_tensor("Wv", [D, D], FP32, kind="ExternalInput").ap()
    wd = nc.dram_tensor("Wd", [D, D], FP32, kind="ExternalInput").ap()
    demb = nc.dram_tensor("demb_win_T", [HD, EW], FP32, kind="ExternalInput").ap()
    dembr = nc.dram_tensor("demb_win_rev_T", [HD, EW], FP32, kind="ExternalInput").ap()
    out = nc.dram_tensor("out", [LQ, D], FP32, kind="ExternalOutput").ap()

    with tile.TileContext(nc) as tc:
        for r in range(repeats):
            qd_dram = nc.dram_tensor(f"qd_scratch{r}", [H, LQ, QD_W], FP16).ap()
            kd_dram = nc.dram_tensor(f"kd_scratch{r}", [H, 8, 128, KD_W], FP16).ap()
            _body(nc, tc, hq, hkv, wq, wk, wv, wd, demb, dembr, out,
                  qd_dram, kd_dram)
    nc.compile()
    return nc


def _body(nc, tc, hq, hkv, wq, wk, wv, wd, demb, dembr, out, qd_dram, kd_dram):
    with tc.tile_pool(name="const", bufs=1) as cp:
        ones_row = cp.tile([1, 64], FP32, tag="ones_row")
        nc.gpsimd.memset(ones_row[:, :], 1.0)

        eT = cp.tile([128, EW], FP32, tag="eT")    # rows 0:64 == 64:128 (replicated)
        erT = cp.tile([128, EW], FP32, tag="erT")
        kT = [cp.tile([128, LK], FP32, tag=f"kT{i}", name=f"kT{i}") for i in range(6)]
        qT = [cp.tile([128, LQ], FP32, tag=f"qT{i}", name=f"qT{i}") for i in range(6)]
        vv = [cp.tile([128, 780], FP32, tag=f"v{i}", name=f"v{i}") for i in range(8)]
        ctxT = [cp.tile([128, LQ], FP32, tag=f"ctxT{i}", name=f"ctxT{i}") for i in range(6)]

        # ---------------- Phase A+B: loads (host pre-transposed) + projections
        with tc.tile_pool(name="xt", bufs=1) as xp:
            xT = [xp.tile([128, LK], FP32, tag=f"xT{i}", name=f"xT{i}") for i in range(6)]
            xqT = [xp.tile([128, LQ], FP32, tag=f"xqT{i}", name=f"xqT{i}") for i in range(6)]

            for half in range(2):
                nc.sync.dma_start(out=eT[64 * half:64 * (half + 1), :], in_=demb[:, :])
                nc.sync.dma_start(out=erT[64 * half:64 * (half + 1), :], in_=dembr[:, :])
            for i in range(6):
                nc.sync.dma_start(out=xT[i][:, :], in_=hkv[128 * i:128 * (i + 1), :])
                nc.sync.dma_start(out=xqT[i][:, :], in_=hq[128 * i:128 * (i + 1), :])

            # projections
            with tc.tile_pool(name="wld", bufs=1) as wp, \
                 tc.tile_pool(name="psB", bufs=2, space="PSUM") as pb:
                for widx, (wdram, dst, rhs_tiles, n_tok) in enumerate((
                        (wk, kT, xT, LK), (wq, qT, xqT, LQ), (wv, None, xT, LK))):
                    wtiles = []
                    for kk in range(6):
                        wt = wp.tile([128, D], FP32, tag=f"w{kk}")
                        nc.sync.dma_start(out=wt[:, :], in_=wdram[128 * kk:128 * (kk + 1), :])
                        wtiles.append(wt)
                    if dst is not None:  # q/k: out is [D, n_tok] transposed
                        for m in range(6):
                            ps = pb.tile([128, LK], FP32, tag="projp")
                            for kk in range(6):
                                for nh in range(n_tok // 512):
                                    nc.tensor.matmul(
                                        ps[:, 512 * nh:512 * (nh + 1)],
                                        wtiles[kk][:, 128 * m:128 * (m + 1)],
                                        rhs_tiles[kk][:, 512 * nh:512 * (nh + 1)],
                                        start=(kk == 0), stop=(kk == 5))
                            nc.scalar.mul(dst[m][:, 0:n_tok], ps[:, 0:n_tok], SCALE)
                    else:  # v: natural [tok, D]
                        for r in range(8):
                            ps = pb.tile([128, D], FP32, tag="projp")
                            for kk in range(6):
                                for o, w in ((0, 512), (512, 256)):
                                    nc.tensor.matmul(
                                        ps[:, o:o + w],
                                        xT[kk][:, 128 * r:128 * (r + 1)],
                                        wtiles[kk][:, o:o + w],
                                        start=(kk == 0), stop=(kk == 5))
                            nc.gpsimd.memset(vv[r][:, :], 1.0)
                            vdst = vv[r][:, 0:D].rearrange("p (h e) -> p h e", e=64)
                            vdst = _strided_view(vdst, [vdst.ap[0], (65, 12), (1, 64)], 0)
                            nc.scalar.copy(vdst, ps[:, 0:D].rearrange(
                                "p (h e) -> p h e", e=64))

        # ---------------- Phase C: per-head attention ----------------
        with tc.tile_pool(name="psC", bufs=2, space="PSUM") as pc, \
             tc.tile_pool(name="psCS", bufs=3, space="PSUM") as pcs, \
             tc.tile_pool(name="psCTX", bufs=1, space="PSUM") as pctx, \
             tc.tile_pool(name="wkC", bufs=3) as wc, \
             tc.tile_pool(name="wkC3", bufs=8) as wc3:
            def _emit_writes(h):
                hc, hp = h // 2, h % 2
                hr = slice(64 * hp, 64 * (hp + 1))
                # kd chunks -> DRAM
                for Jp in range(4):
                    kd_sb = wc.tile([128, 2 * KD_W], FP16, tag="kd_sb")
                    for half in range(2):
                        J = 2 * Jp + half
                        w0 = 896 - 128 * J
                        kdp = pc.tile([128, KD_W], FP32, tag="kdqd")
                        lhsT = kT[hc][hr, 128 * J:128 * (J + 1)]
                        nc.tensor.matmul(kdp[:, 0:512], lhsT, eT[hr, w0:w0 + 512],
                                         start=True, stop=True)
                        nc.tensor.matmul(kdp[:, 512:KD_W], lhsT,
                                         eT[hr, w0 + 512:w0 + KD_W],
                                         start=True, stop=True)
                        nc.scalar.copy(kd_sb[:, KD_W * half:KD_W * (half + 1)],
                                       kdp[:, 0:KD_W])
                    kdw = _strided_view(kd_dram[h, 2 * Jp].unsqueeze(1),
                                        [(KD_W, 128), (128 * KD_W, 2), (1, KD_W)], 0)
                    nc.sync.dma_start(out=kdw, in_=kd_sb[:, :].rearrange(
                        "p (two w) -> p two w", two=2))

                # qd chunks -> DRAM (fp16, reversed window)
                for Ip in range(2):
                    qd_sb = wc.tile([128, 2, 1152], FP16, tag="qd_sb")
                    for half in range(2):
                        I = 2 * Ip + half
                        c0 = 384 - 128 * I
                        lhsT = qT[hc][hr, 128 * I:128 * (I + 1)]
                        qdpA = pc.tile([128, KD_W], FP32, tag="kdqd")
                        for o, w in ((0, 512), (512, 128)):
                            nc.tensor.matmul(qdpA[:, o:o + w], lhsT,
                                             erT[hr, c0 + o:c0 + o + w],
                                             start=True, stop=True)
                        nc.vector.tensor_copy(qd_sb[:, half, 0:KD_W], qdpA[:, :])
                        qdpB = pc.tile([128, 512], FP32, tag="kdqd")
                        nc.tensor.matmul(qdpB[:, :], lhsT,
                                         erT[hr, c0 + KD_W:c0 + KD_W + 512],
                                         start=True, stop=True)
                        nc.vector.tensor_copy(qd_sb[:, half, KD_W:1152], qdpB[:, :])
                    # rows of the I-pair: row step 1536, I-step = 128*1536 - 128
                    c0p = 384 - 256 * Ip
                    qdw = _strided_view(
                        qd_dram[h, 256 * Ip:256 * Ip + 128, c0p:c0p + 1152]
                        .unsqueeze(1),
                        [(QD_W, 128), (128 * QD_W - 128, 2), (1, 1152)], 0)
                    nc.sync.dma_start(out=qdw, in_=qd_sb[:, :, :].rearrange(
                        "p a b -> p (a b)").rearrange("p (a b) -> p a b", a=2))


            def _emit_scores(h):
                hc, hp = h // 2, h % 2
                hr = slice(64 * hp, 64 * (hp + 1))
                # scores + PV
                ctxp = pctx.tile([65, LQ], FP32, tag="ctxp")
                for J in range(8):
                    # early independent reads: rel_q (xbar transpose) + rel_k
                    rq_sb = wc3.tile([128, LQ], FP16, tag="rq_sb")
                    qdv = _strided_view(qd_dram[h], [(QD_W - 1, LQ), (1, 128)],
                                        512 + 128 * J)
                    nc.scalar.dma_start(out=rq_sb[:, :], in_=qdv, transpose=True)
                    if J % 2 == 0:
                        rk2_sb = wc3.tile([128, 2, LQ], FP16, tag="rk2_sb")
                        kdv = _strided_view(
                            kd_dram[h, J].unsqueeze(1),
                            [(KD_W - 1, 128), (128 * KD_W, 2), (1, LQ)], 127)
                        nc.sync.dma_start(out=rk2_sb[:, :, :], in_=kdv)
                    rk_sb = rk2_sb[:, J % 2, :]
                    rel_sb = wc3.tile([128, LQ], FP16, tag="rel_sb")
                    nc.gpsimd.tensor_tensor(rel_sb[:, :], rq_sb[:, :], rk_sb, ADD)
                    csp = pcs.tile([128, LQ], FP32, tag="csp")
                    nc.tensor.matmul(csp[:, :], kT[hc][hr, 128 * J:128 * (J + 1)],
                                     qT[hc][hr, :], start=True, stop=True)
                    s_sb = wc3.tile([128, LQ], FP32, tag="s_sb")
                    nc.vector.tensor_tensor(s_sb[:, :], csp[:, :], rel_sb[:, :], ADD)
                    p_sb = wc3.tile([128, LQ], FP32, tag="p_sb")
                    nc.scalar.activation(p_sb[:, :], s_sb[:, :], EXP)
                    # PV (rows 0:64) + denominators (rows 64:128) as two
                    # col-tiled concurrent matmuls sharing the rhs stream
                    nc.tensor.matmul(ctxp[:, :], vv[J][:, 65 * h:65 * h + 65],
                                     p_sb[:, :], start=(J == 0), stop=(J == 7))

                # normalize: ctxT_h = ctx' * (1/denom) broadcast over partitions
                recip = wc.tile([1, LQ], FP32, tag="recip")
                nc.vector.reciprocal(recip[:, :], ctxp[64:65, :])
                bcp = pcs.tile([64, LQ], FP32, tag="csp")
                nc.tensor.matmul(bcp[:, :], ones_row[:, :], recip[:, :],
                                 start=True, stop=True)
                bc_sb = wc.tile([64, LQ], FP32, tag="bc_sb")
                nc.scalar.copy(bc_sb[:, :], bcp[:, :])
                nc.vector.tensor_tensor(ctxT[hc][hr, :], ctxp[0:64, :],
                                        bc_sb[:, :], MULT)


            for h in range(H + 1):
                if h < H:
                    _emit_writes(h)
                if h >= 1:
                    _emit_scores(h - 1)

        # ---------------- Phase D: output projection ----------------
        with tc.tile_pool(name="wdld", bufs=1) as dp, \
             tc.tile_pool(name="psD", bufs=2, space="PSUM") as pd, \
             tc.tile_pool(name="oD", bufs=2) as od:
            wdt = []
            for kk in range(6):
                wt = dp.tile([128, D], FP32, tag=f"wd{kk}")
                nc.sync.dma_start(out=wt[:, :], in_=wd[128 * kk:128 * (kk + 1), :])
                wdt.append(wt)
            for lc in range(4):
                ps = pd.tile([128, D], FP32, tag="outp")
                for kk in range(6):
                    for o, w in ((0, 512), (512, 256)):
                        nc.tensor.matmul(ps[:, o:o + w],
                                         ctxT[kk][:, 128 * lc:128 * (lc + 1)],
                                         wdt[kk][:, o:o + w],
                                         start=(kk == 0), stop=(kk == 5))
                o_sb = od.tile([128, D], FP32, tag="o_sb")
                nc.scalar.copy(o_sb[:, :], ps[:, :])
                nc.sync.dma_start(out=out[128 * lc:128 * (lc + 1), :], in_=o_sb[:, :])


_NC_CACHE = None


def _get_nc():
    global _NC_CACHE
    if _NC_CACHE is None:
        _NC_CACHE = build_nc()
    return _NC_CACHE


def make_in_maps(hidden_states, Wq, Wk, Wv, Wd, dist_emb):
    E = np.ascontiguousarray(np.asarray(dist_emb, np.float32))
    in_maps = []
    for core in range(N_CORES):
        b, lh = core // 2, core % 2
        l0 = LQ * lh
        win = np.zeros((EW, HD), np.float32)
        n = min(EW, E.shape[0] - l0)
        win[:n] = E[l0:l0 + n]
        wins = win * np.float32(SCALE)
        in_maps.append({
            "hidden_q_T": np.ascontiguousarray(hidden_states[b, l0:l0 + LQ].T),
            "hidden_kv_T": np.ascontiguousarray(hidden_states[b].T),
            "Wq": np.ascontiguousarray(Wq), "Wk": np.ascontiguousarray(Wk),
            "Wv": np.ascontiguousarray(Wv), "Wd": np.ascontiguousarray(Wd),
            "demb_win_T": np.ascontiguousarray(wins.T),
            "demb_win_rev_T": np.ascontiguousarray(wins[::-1].T),
        })
    return in_maps


def run(inputs, trace=False):
    """Returns (full_output [4,1024,768], BassKernelResults)."""
    nc = _get_nc()
    in_maps = make_in_maps(inputs["hidden_states"], inputs["Wq"], inputs["Wk"],
                           inputs["Wv"], inputs["Wd"], inputs["dist_emb"])
    res = run_bass_kernel_spmd(nc, in_maps, list(range(N_CORES)), trace=trace)
    full = np.zeros((4, LK, D), np.float32)
    for core in range(N_CORES):
        b, lh = core // 2, core % 2
        full[b, LQ * lh:LQ * (lh + 1)] = res.results[core]["out"]
    return full, res


def kernel(**inputs):
    full, _ = run(inputs, trace=False)
    return full


if __name__ == "__main__":
    # quick self-build check
    nc = build_nc()
    print("built ok")



# revision 28
# speedup vs baseline: 15.0677x; 15.0677x over previous
"""Trainium2 Bass kernel for relative-position attention (nn_AttentionMechanism).

Math (per batch b):
  q,k,v = h@Wq, h@Wk, h@Wv  (biases are zero in this problem)
  scores[l,r] = (q[l].k[r] + q[l].E[l-r+1023] + k[r].E[l-r+1023]) / sqrt(64)
  out = softmax(scores) @ v @ Wd

Sharding: 8 cores = (batch b in 0..3) x (query half lh in 0..1).
Each core computes out rows [lh*512, lh*512+512) for batch b.

v3: all matmuls bf16 (4x PE throughput vs fp32); the relative-position shear
round-trips through DRAM in fp8e4 (half the fp16 baseline's DMA bytes):
  - kd[r,j] = k[r].E_win[j] per head -> DRAM fp8, read back with row stride
    639 (music-transformer trick) -> rel_k^T.
  - qd[l,u] = q[l].E_win_rev[u] for a HEAD PAIR interleaved as 2-byte units
    (h0,h1 fp8 pairs); the xbar transpose-DMA (2-byte elems) shear-reads it
    -> rel_q^T for both heads in one transfer.
  - rel_q + rel_k summed by DVE directly INTO PSUM; the content-score matmul
    accumulates on top (start=False); exp reads PSUM once.
Softmax denominators come free as a 65th ones-column in v (PV matmul).

Structure: two pipelined phases with dedicated PSUM pools (PSUM = 8 banks is
the scarce resource). Phase W: per-pair k/q projection feeding kd/qd
generation; PSUM pools are split BY CONSUMER ENGINE (kd+proj copies on Act,
qd copies on DVE) so neither copy stream stalls the other through the
buffer rotation. Phase S: shear reads + score/softmax/PV (Act/DVE-bound;
v-projection folded in here where the PE is otherwise idle). Shear reads for
the first two pairs are prefetched at the end of phase W so phase S starts
hot. Output projection last.
"""

import sys

sys.path.insert(0, "/opt/trn_rl_repo")

import numpy as np

import concourse.bass as bass
import concourse.mybir as mybir
import concourse.tile as tile
from concourse import bacc
from concourse.bass_utils import run_bass_kernel_spmd

FP32 = mybir.dt.float32
BF16 = mybir.dt.bfloat16
FP16 = mybir.dt.float16
FP8 = mybir.dt.float8e4
ADD = mybir.AluOpType.add
MULT = mybir.AluOpType.mult
EXP = mybir.ActivationFunctionType.Exp

N_CORES = 8
D, H, HD = 768, 12, 64
G = 6              # head pairs
LQ, LK = 512, 1024
EW = 1536          # E window rows per core
KD_W = 640         # kd chunk width
QD_W = 1536        # qd row stride (in 2-byte units)
SCALE = 0.35355339059327373  # 8**-0.5 applied to q,k AND E => all terms get /8


def _strided_view(ap, dims, extra_offset):
    """Return a copy of `ap` with its [step,count] pairs and offset replaced."""
    v = ap.copy()
    a = v.ap
    assert len(a) == len(dims), (a, dims)
    for i, d in enumerate(dims):
        a[i] = list(d)
    v.ap = a
    v.offset = v.offset + extra_offset
    return v


def build_nc(repeats=1):
    nc = bacc.Bacc("TRN2", target_bir_lowering=False, debug=False,
                   num_devices=N_CORES)

    hq = nc.dram_tensor("hidden_q_T", [D, LQ], BF16, kind="ExternalInput").ap()
    hkv = nc.dram_tensor("hidden_kv_T", [D, LK], BF16, kind="ExternalInput").ap()
    wq = nc.dram_tensor("Wq", [D, D], BF16, kind="ExternalInput").ap()
    wk = nc.dram_tensor("Wk", [D, D], BF16, kind="ExternalInput").ap()
    wv = nc.dram_tensor("Wv", [D, D], BF16, kind="ExternalInput").ap()
    wd = nc.dram_tensor("Wd", [D, D], BF16, kind="ExternalInput").ap()
    demb = nc.dram_tensor("demb_win_T", [HD, EW], BF16, kind="ExternalInput").ap()
    ident = nc.dram_tensor("ident8", [128, 128], FP8, kind="ExternalInput").ap()
    dembr = nc.dram_tensor("demb_win_rev_T", [HD, EW], BF16, kind="ExternalInput").ap()
    out = nc.dram_tensor("out", [LQ, D], FP32, kind="ExternalOutput").ap()

    with tile.TileContext(nc) as tc:
        for r in range(repeats):
            kd_dram = nc.dram_tensor(f"kd_scratch{r}", [G, 8, 2, 128, KD_W],
                                     FP8).ap()
            qd_dram = nc.dram_tensor(f"qd_scratch{r}", [G, LQ, QD_W],
                                     FP16).ap()
            _body(nc, tc, hq, hkv, wq, wk, wv, wd, demb, dembr, ident, out,
                  qd_dram, kd_dram)
    nc.compile()
    return nc


def _body(nc, tc, hq, hkv, wq, wk, wv, wd, demb, dembr, ident, out,
          qd_dram, kd_dram):
    with tc.tile_pool(name="const", bufs=1) as cp, \
         tc.tile_pool(name="rqP", bufs=25) as rqp, \
         tc.tile_pool(name="rkP", bufs=13) as rkp:
        ones_row = cp.tile([1, 64], BF16, tag="ones_row")
        nc.gpsimd.memset(ones_row[:, :], 1.0)

        eT = cp.tile([128, EW], BF16, tag="eT")    # rows 0:64 == 64:128
        erT = cp.tile([128, EW], BF16, tag="erT")
        kT = cp.tile([128, G, LK], BF16, tag="kT")
        qT = cp.tile([128, G, LQ], BF16, tag="qT")
        vv = cp.tile([128, 8, 780], BF16, tag="vv")
        ctxT = cp.tile([128, G, LQ], BF16, tag="ctxT")
        xT = cp.tile([128, 6, LK], BF16, tag="xT")
        xqT = cp.tile([128, 6, LQ], BF16, tag="xqT")
        wvt = cp.tile([128, 6, D], BF16, tag="wvt")
        id8 = cp.tile([128, 128], FP8, tag="id8")
        nc.sync.dma_start(out=id8[:, :], in_=ident[:, :])
        nc.gpsimd.memset(vv[:, :, :], 1.0)

        # ------------- loads (host pre-transposed, bf16); proj deps first ----
        nc.sync.dma_start(out=xT[:, :, :],
                          in_=hkv.rearrange("(kk p) t -> p kk t", p=128))

        wdt = cp.tile([128, 6, D], BF16, tag="wd")
        nc.sync.dma_start(out=wdt[:, :, :],
                          in_=wd.rearrange("(kk p) t -> p kk t", p=128))

        rq_all, rk_all = {}, {}

        def _emit_reads(g):
            rq_sb, rk_sb = [], []
            for J in range(8):
                rq = rqp.tile([128, LQ], FP16, tag="rq_sb")
                qdv = _strided_view(qd_dram[g][0:512, 0:128],
                                    [(QD_W - 1, 512), (1, 128)],
                                    512 + 128 * J)
                nc.sync.dma_start(out=rq[:, :], in_=qdv, transpose=True)
                rq_sb.append(rq)
            for Jp in range(4):
                rk = rkp.tile([128, 4, 512], FP8, tag="rk_sb")
                kdv = kd_dram[g, 2 * Jp:2 * Jp + 2, :, 0:128, 0:512]
                kdv = kdv.transpose([2, 0, 1, 3]).rearrange(
                    "p a b x -> p (a b) x")
                kdv = _strided_view(
                    kdv,
                    [(KD_W - 1, 128), (128 * KD_W, 4), (1, 512)],
                    127)
                nc.sync.dma_start(out=rk[:, :, :], in_=kdv)
                rk_sb.append(rk)
            rq_all[g] = rq_sb
            rk_all[g] = rk_sb

        # ---------------- Phase W: k/q proj + kd/qd shear writes ----------
        with tc.tile_pool(name="wkq", bufs=1) as wkqp, \
             tc.tile_pool(name="psKD", bufs=3, space="PSUM") as pkd, \
             tc.tile_pool(name="psQD", bufs=2, space="PSUM") as pqd, \
             tc.tile_pool(name="kdsb", bufs=2) as kdp_pool, \
             tc.tile_pool(name="qdsb", bufs=2) as qdp_pool:
            wkt = wkqp.tile([128, 6, D], BF16, tag="wk")
            wqt = wkqp.tile([128, 6, D], BF16, tag="wq")
            nc.sync.dma_start(out=wkt[:, :, :],
                              in_=wk.rearrange("(kk p) t -> p kk t", p=128))
            nc.sync.dma_start(out=xqT[:, :, :],
                              in_=hq.rearrange("(kk p) t -> p kk t", p=128))
            nc.sync.dma_start(out=wqt[:, :, :],
                              in_=wq.rearrange("(kk p) t -> p kk t", p=128))
            for half in range(2):
                nc.sync.dma_start(out=eT[64 * half:64 * (half + 1), :],
                                  in_=demb[:, :])
                nc.sync.dma_start(out=erT[64 * half:64 * (half + 1), :],
                                  in_=dembr[:, :])
            nc.sync.dma_start(out=wvt[:, :, :],
                              in_=wv.rearrange("(kk p) t -> p kk t", p=128))

            def _proj_kq(g):
                for nh in range(2):
                    ps = pkd.tile([128, 512], FP32, tag="pkd")
                    for kk in range(6):
                        nc.tensor.matmul(ps[:, :],
                                         wkt[:, kk, 128 * g:128 * (g + 1)],
                                         xT[:, kk, 512 * nh:512 * (nh + 1)],
                                         start=(kk == 0), stop=(kk == 5))
                    nc.scalar.mul(kT[:, g, 512 * nh:512 * (nh + 1)],
                                  ps[:, :], SCALE)
                ps = pkd.tile([128, 512], FP32, tag="pkd")
                for kk in range(6):
                    nc.tensor.matmul(ps[:, :],
                                     wqt[:, kk, 128 * g:128 * (g + 1)],
                                     xqT[:, kk, :], start=(kk == 0),
                                     stop=(kk == 5))
                nc.scalar.mul(qT[:, g, :], ps[:, :], SCALE)

            def _emit_write_slot(g, J, kd_sb, qd_sb):
                for h01 in range(2):
                    hr = slice(64 * h01, 64 * (h01 + 1))
                    w0 = 896 - 128 * J
                    kdp = pkd.tile([128, KD_W], FP32, tag="pkd")
                    lhsT = kT[hr, g, 128 * J:128 * (J + 1)]
                    nc.tensor.matmul(kdp[:, 0:512], lhsT, eT[hr, w0:w0 + 512],
                                     start=True, stop=True)
                    nc.tensor.matmul(kdp[:, 512:KD_W], lhsT,
                                     eT[hr, w0 + 512:w0 + KD_W],
                                     start=True, stop=True)
                    nc.scalar.copy(kd_sb[:, J, h01, :], kdp[:, :])
                # one qd unit per slot, DVE-consumed from its own pool
                h01, I = J % 2, J // 2
                c0 = 384 - 128 * I
                Ip, half = I // 2, I % 2
                hr = slice(64 * h01, 64 * (h01 + 1))
                lhsT = qT[hr, g, 128 * I:128 * (I + 1)]
                qdpA = pqd.tile([128, 512], FP32, tag="pqd")
                nc.tensor.matmul(qdpA[:, :], lhsT, erT[hr, c0:c0 + 512],
                                 start=True, stop=True)
                nc.vector.tensor_copy(qd_sb[:, Ip, half, 0:512, h01],
                                      qdpA[:, :])
                qdpB = pqd.tile([128, 512], FP32, tag="pqd")
                nc.tensor.matmul(qdpB[:, 0:128], lhsT,
                                 erT[hr, c0 + 512:c0 + 640],
                                 start=True, stop=True)
                nc.tensor.matmul(qdpB[:, 128:512], lhsT,
                                 erT[hr, c0 + 640:c0 + 1024],
                                 start=True, stop=True)
                nc.vector.tensor_copy(
                    qd_sb[:, Ip, half, 512:1024, h01], qdpB[:, :])
                qdpC = pqd.tile([128, 128], FP32, tag="pqd")
                nc.tensor.matmul(qdpC[:, :], lhsT,
                                 erT[hr, c0 + 1024:c0 + 1152],
                                 start=True, stop=True)
                nc.vector.tensor_copy(
                    qd_sb[:, Ip, half, 1024:1152, h01], qdpC[:, :])
                # staged DRAM writes
                if J % 2 == 1:
                    Jp = J // 2
                    dst = kd_dram[g, 2 * Jp:2 * Jp + 2].transpose([2, 0, 1, 3])
                    nc.sync.dma_start(
                        out=dst, in_=kd_sb[:, 2 * Jp:2 * Jp + 2, :, :])
                if J == 7:
                    qd8 = qd_dram[g].bitcast(FP8)  # [512, 3072] fp8
                    dst = qd8[0:128, 0:2].unsqueeze(2).unsqueeze(3)
                    dst = _strided_view(
                        dst,
                        [(2 * QD_W, 128), (2 * (256 * QD_W - 256), 2),
                         (2 * (128 * QD_W - 128), 2), (1, 2304)],
                        2 * 384)
                    nc.sync.dma_start(
                        out=dst,
                        in_=qd_sb[:, :, :, :, :].rearrange(
                            "p a b u h -> p a b (u h)"))

            _proj_kq(0)
            for g in range(G):
                if g + 1 < G:
                    _proj_kq(g + 1)
                kd_sb = kdp_pool.tile([128, 8, 2, KD_W], FP8, tag="kd_sb")
                qd_sb = qdp_pool.tile([128, 2, 2, 1152, 2], FP8, tag="qd_sb")
                for J in range(8):
                    _emit_write_slot(g, J, kd_sb, qd_sb)
                # prefetch first shear reads late in W so phase S starts hot
                if g == 3:
                    _emit_reads(0)
                if g == 5:
                    _emit_reads(1)

        # ---------------- Phase S: shear reads + scores + PV --------------
        with tc.tile_pool(name="psS", bufs=4, space="PSUM") as pcs, \
             tc.tile_pool(name="psV", bufs=2, space="PSUM") as pv, \
             tc.tile_pool(name="psCTX", bufs=2, space="PSUM") as pctx, \
             tc.tile_pool(name="wkC", bufs=3) as wc, \
             tc.tile_pool(name="pP", bufs=6) as pp:

            def _emit_score_slot(g, J, ctxp):
                rq8 = rq_all[g][J][:, :].bitcast(FP8).rearrange(
                    "p (l two) -> p l two", two=2)
                for h01 in range(2):
                    h = 2 * g + h01
                    hr = slice(64 * h01, 64 * (h01 + 1))
                    relp = pcs.tile([128, LQ], FP32, tag="pcs")
                    rkv = rk_all[g][J // 2][:, 2 * (J % 2) + h01, :]
                    nc.vector.tensor_tensor(relp[:, :], rq8[:, :, h01],
                                            rkv, ADD)
                    nc.tensor.matmul(relp[:, :],
                                     kT[hr, g, 128 * J:128 * (J + 1)],
                                     qT[hr, g, :], start=False, stop=True)
                    p_sb = pp.tile([128, LQ], BF16, tag="p_sb")
                    nc.scalar.activation(p_sb[:, :], relp[:, :], EXP)
                    nc.tensor.matmul(ctxp[h01][:, :],
                                     vv[:, J, 65 * h:65 * h + 65],
                                     p_sb[:, :], start=(J == 0), stop=(J == 7))

            def _emit_normalize(g, ctxp):
                # normalize: ctxT_h = ctx' * (1/denom) broadcast over partitions
                for h01 in range(2):
                    hr = slice(64 * h01, 64 * (h01 + 1))
                    recip = wc.tile([1, LQ], BF16, tag="recip")
                    with nc.allow_low_precision(
                            reason="bf16 denom reciprocal; 0.4% scale error ok"):
                        nc.vector.reciprocal(recip[:, :], ctxp[h01][64:65, :])
                    bcp = pcs.tile([64, LQ], FP32, tag="pcs")
                    nc.tensor.matmul(bcp[:, :], ones_row[:, :], recip[:, :],
                                     start=True, stop=True)
                    bc_sb = wc.tile([64, LQ], FP32, tag="bc_sb")
                    nc.scalar.copy(bc_sb[:, :], bcp[:, :])
                    nc.vector.tensor_tensor(ctxT[hr, g, :], ctxp[h01][0:64, :],
                                            bc_sb[:, :], MULT)

            for g in range(G):
                if g + 2 < G:
                    _emit_reads(g + 2)
                ctxp = [pctx.tile([65, LQ], FP32, tag="ctxp",
                                  name=f"ctxp{i}") for i in range(2)]
                for r in range(8):
                    psv = pv.tile([128, 128], FP32, tag="pv")
                    for kk in range(6):
                        nc.tensor.matmul(psv[:, :],
                                         xT[:, kk, 128 * r:128 * (r + 1)],
                                         wvt[:, kk, 128 * g:128 * (g + 1)],
                                         start=(kk == 0), stop=(kk == 5))
                    vdst = vv[:, r, 130 * g:130 * g + 130].rearrange(
                        "p (h e) -> p h e", e=65)[:, :, 0:64]
                    vsrc = psv[:, :].rearrange("p (h e) -> p h e", e=64)
                    nc.scalar.copy(vdst, vsrc)
                for J in range(8):
                    _emit_score_slot(g, J, ctxp)
                _emit_normalize(g, ctxp)
                del rq_all[g], rk_all[g]

        # ---------------- Phase D: output projection ----------------
        with tc.tile_pool(name="psD", bufs=2, space="PSUM") as pd, \
             tc.tile_pool(name="oD", bufs=2) as od:
            for lc in range(4):
                ps = pd.tile([128, D], FP32, tag="outp")
                for kk in range(6):
                    for o, w in ((0, 512), (512, 256)):
                        nc.tensor.matmul(ps[:, o:o + w],
                                         ctxT[:, kk, 128 * lc:128 * (lc + 1)],
                                         wdt[:, kk, o:o + w],
                                         start=(kk == 0), stop=(kk == 5))
                o_sb = od.tile([128, D], FP32, tag="o_sb")
                nc.scalar.copy(o_sb[:, :], ps[:, :])
                nc.sync.dma_start(out=out[128 * lc:128 * (lc + 1), :], in_=o_sb[:, :])


_NC_CACHE = None


def _get_nc():
    global _NC_CACHE
    if _NC_CACHE is None:
        _NC_CACHE = build_nc()
    return _NC_CACHE


def make_in_maps(hidden_states, Wq, Wk, Wv, Wd, dist_emb):
    import ml_dtypes
    bf16 = ml_dtypes.bfloat16
    E = np.ascontiguousarray(np.asarray(dist_emb, np.float32))
    h32 = np.asarray(hidden_states, np.float32)
    Wqb = np.ascontiguousarray(np.asarray(Wq, np.float32).astype(bf16))
    Wkb = np.ascontiguousarray(np.asarray(Wk, np.float32).astype(bf16))
    Wvb = np.ascontiguousarray(np.asarray(Wv, np.float32).astype(bf16))
    Wdb = np.ascontiguousarray(np.asarray(Wd, np.float32).astype(bf16))
    in_maps = []
    for core in range(N_CORES):
        b, lh = core // 2, core % 2
        l0 = LQ * lh
        win = np.zeros((EW, HD), np.float32)
        n = min(EW, E.shape[0] - l0)
        win[:n] = E[l0:l0 + n]
        wins = win * np.float32(SCALE)
        in_maps.append({
            "ident8": np.eye(128, dtype=ml_dtypes.float8_e4m3),
            "hidden_q_T": np.ascontiguousarray(h32[b, l0:l0 + LQ].T.astype(bf16)),
            "hidden_kv_T": np.ascontiguousarray(h32[b].T.astype(bf16)),
            "Wq": Wqb, "Wk": Wkb, "Wv": Wvb, "Wd": Wdb,
            "demb_win_T": np.ascontiguousarray(wins.T.astype(bf16)),
            "demb_win_rev_T": np.ascontiguousarray(wins[::-1].T.astype(bf16)),
        })
    return in_maps


def run(inputs, trace=False):
    """Returns (full_output [4,1024,768], BassKernelResults)."""
    nc = _get_nc()
    in_maps = make_in_maps(inputs["hidden_states"], inputs["Wq"], inputs["Wk"],
                           inputs["Wv"], inputs["Wd"], inputs["dist_emb"])
    res = run_bass_kernel_spmd(nc, in_maps, list(range(N_CORES)), trace=trace)
    full = np.zeros((4, LK, D), np.float32)
    for core in range(N_CORES):
        b, lh = core // 2, core % 2
        full[b, LQ * lh:LQ * (lh + 1)] = res.results[core]["out"]
    return full, res


def kernel(**inputs):
    full, _ = run(inputs, trace=False)
    return full


if __name__ == "__main__":
    # quick self-build check
    nc = build_nc()
    print("built ok")


# revision 33
# speedup vs baseline: 17.1504x; 1.1382x over previous
"""Trainium2 Bass kernel for relative-position attention (nn_AttentionMechanism).

Math (per batch b):
  q,k,v = h@Wq, h@Wk, h@Wv  (biases are zero in this problem)
  scores[l,r] = (q[l].k[r] + q[l].E[l-r+1023] + k[r].E[l-r+1023]) / sqrt(64)
  out = softmax(scores) @ v @ Wd

Sharding: 8 cores = (batch b in 0..3) x (query half lh in 0..1).
Each core computes out rows [lh*512, lh*512+512) for batch b.

v3: all matmuls bf16 (4x PE throughput vs fp32); the relative-position shear
round-trips through DRAM in fp8e4 (half the fp16 baseline's DMA bytes):
  - kd[r,j] = k[r].E_win[j] per head -> DRAM fp8, read back with row stride
    639 (music-transformer trick) -> rel_k^T.
  - qd[l,u] = q[l].E_win_rev[u] for a HEAD PAIR interleaved as 2-byte units
    (h0,h1 fp8 pairs); the xbar transpose-DMA (2-byte elems) shear-reads it
    -> rel_q^T for both heads in one transfer.
  - rel_q + rel_k summed by DVE directly INTO PSUM; the content-score matmul
    accumulates on top (start=False); exp reads PSUM once.
Softmax denominators come free as a 65th ones-column in v (PV matmul).

Structure: two pipelined phases with dedicated PSUM pools (PSUM = 8 banks is
the scarce resource). Phase W: per-pair k/q projection feeding kd/qd
generation; PSUM pools are split BY CONSUMER ENGINE (kd+proj copies on Act,
qd copies on DVE) so neither copy stream stalls the other through the
buffer rotation. Phase S: shear reads + score/softmax/PV (Act/DVE-bound;
v-projection folded in here where the PE is otherwise idle). Shear reads for
the first two pairs are prefetched at the end of phase W so phase S starts
hot. Output projection last.
"""

import sys

sys.path.insert(0, "/opt/trn_rl_repo")

import numpy as np

import concourse.bass as bass
import concourse.mybir as mybir
import concourse.tile as tile
from concourse import bacc
from concourse.bass_utils import run_bass_kernel_spmd

FP32 = mybir.dt.float32
BF16 = mybir.dt.bfloat16
FP16 = mybir.dt.float16
FP8 = mybir.dt.float8e4
ADD = mybir.AluOpType.add
MULT = mybir.AluOpType.mult
EXP = mybir.ActivationFunctionType.Exp

N_CORES = 8
D, H, HD = 768, 12, 64
G = 6              # head pairs
LQ, LK = 512, 1024
EW = 1536          # E window rows per core
KD_W = 640         # kd chunk width
QD_W = 1536        # qd row stride (in 2-byte units)
SCALE = 0.35355339059327373  # 8**-0.5 applied to q,k AND E => all terms get /8


def _strided_view(ap, dims, extra_offset):
    """Return a copy of `ap` with its [step,count] pairs and offset replaced."""
    v = ap.copy()
    a = v.ap
    assert len(a) == len(dims), (a, dims)
    for i, d in enumerate(dims):
        a[i] = list(d)
    v.ap = a
    v.offset = v.offset + extra_offset
    return v


def build_nc(repeats=1):
    nc = bacc.Bacc("TRN2", target_bir_lowering=False, debug=False,
                   num_devices=N_CORES)

    hq = nc.dram_tensor("hidden_q_T", [D, LQ], BF16, kind="ExternalInput").ap()
    hkv = nc.dram_tensor("hidden_kv_T", [D, LK], BF16, kind="ExternalInput").ap()
    wq = nc.dram_tensor("Wq", [D, D], BF16, kind="ExternalInput").ap()
    wk = nc.dram_tensor("Wk", [D, D], BF16, kind="ExternalInput").ap()
    wv = nc.dram_tensor("Wv", [D, D], BF16, kind="ExternalInput").ap()
    wd = nc.dram_tensor("Wd", [D, D], BF16, kind="ExternalInput").ap()
    demb = nc.dram_tensor("demb_win_T", [HD, EW], BF16, kind="ExternalInput").ap()
    dembr = nc.dram_tensor("demb_win_rev_T", [HD, EW], BF16, kind="ExternalInput").ap()
    out = nc.dram_tensor("out", [LQ, D], FP32, kind="ExternalOutput").ap()

    with tile.TileContext(nc) as tc:
        for r in range(repeats):
            kd_dram = nc.dram_tensor(f"kd_scratch{r}", [G, 8, 2, 128, KD_W],
                                     FP8).ap()
            qd_dram = nc.dram_tensor(f"qd_scratch{r}", [G, LQ, QD_W],
                                     FP16).ap()
            _body(nc, tc, hq, hkv, wq, wk, wv, wd, demb, dembr, out,
                  qd_dram, kd_dram)
    nc.compile()
    return nc


def _body(nc, tc, hq, hkv, wq, wk, wv, wd, demb, dembr, out,
          qd_dram, kd_dram):
    with tc.tile_pool(name="const", bufs=1) as cp, \
         tc.tile_pool(name="rqP", bufs=25) as rqp, \
         tc.tile_pool(name="rkP", bufs=13) as rkp:
        ones_row = cp.tile([1, 64], BF16, tag="ones_row")
        nc.gpsimd.memset(ones_row[:, :], 1.0)

        eT = cp.tile([128, EW], BF16, tag="eT")    # rows 0:64 == 64:128
        erT = cp.tile([128, EW], BF16, tag="erT")
        kT = cp.tile([128, G, LK], BF16, tag="kT")
        qT = cp.tile([128, G, LQ], BF16, tag="qT")
        vv = cp.tile([128, 8, 780], BF16, tag="vv")
        ctxT = cp.tile([128, G, LQ], BF16, tag="ctxT")
        xT = cp.tile([128, 6, LK], BF16, tag="xT")
        xqT = cp.tile([128, 6, LQ], BF16, tag="xqT")
        wvt = cp.tile([128, 6, D], BF16, tag="wvt")
        nc.gpsimd.memset(vv[:, :, :], 1.0)

        # ------------- loads (host pre-transposed, bf16); proj deps first ----
        # split loads so proj(0)'s first kk-chunks can start early
        xv = hkv.rearrange("(kk p) t -> p kk t", p=128)
        nc.sync.dma_start(out=xT[:, 0:2, :], in_=xv[:, 0:2, :])

        wdt = cp.tile([128, 6, D], BF16, tag="wd")

        rq_all, rk_all = {}, {}

        def _emit_reads(g):
            rq_sb, rk_sb = [], []
            for J in range(8):
                rq = rqp.tile([128, LQ], FP16, tag="rq_sb")
                qdv = _strided_view(qd_dram[g][0:512, 0:128],
                                    [(QD_W - 1, 512), (1, 128)],
                                    512 + 128 * J)
                nc.sync.dma_start(out=rq[:, :], in_=qdv, transpose=True)
                rq_sb.append(rq)
            for Jp in range(4):
                rk = rkp.tile([128, 4, 512], FP8, tag="rk_sb")
                kdv = kd_dram[g, 2 * Jp:2 * Jp + 2, :, 0:128, 0:512]
                kdv = kdv.transpose([2, 0, 1, 3]).rearrange(
                    "p a b x -> p (a b) x")
                kdv = _strided_view(
                    kdv,
                    [(KD_W - 1, 128), (128 * KD_W, 4), (1, 512)],
                    127)
                nc.sync.dma_start(out=rk[:, :, :], in_=kdv)
                rk_sb.append(rk)
            rq_all[g] = rq_sb
            rk_all[g] = rk_sb

        # ---------------- Phase W: k/q proj + kd/qd shear writes ----------
        with tc.tile_pool(name="wkq", bufs=1) as wkqp, \
             tc.tile_pool(name="psKD", bufs=3, space="PSUM") as pkd, \
             tc.tile_pool(name="psQD", bufs=2, space="PSUM") as pqd, \
             tc.tile_pool(name="kdsb", bufs=2) as kdp_pool, \
             tc.tile_pool(name="qdsb", bufs=2) as qdp_pool:
            wkt = wkqp.tile([128, 6, D], BF16, tag="wk")
            wqt = wkqp.tile([128, 6, D], BF16, tag="wq")
            wkv = wk.rearrange("(kk p) t -> p kk t", p=128)
            nc.sync.dma_start(out=wkt[:, 0:2, :], in_=wkv[:, 0:2, :])
            nc.sync.dma_start(out=xT[:, 2:6, :], in_=xv[:, 2:6, :])
            nc.sync.dma_start(out=wkt[:, 2:6, :], in_=wkv[:, 2:6, :])
            nc.sync.dma_start(out=xqT[:, :, :],
                              in_=hq.rearrange("(kk p) t -> p kk t", p=128))
            nc.sync.dma_start(out=wqt[:, :, :],
                              in_=wq.rearrange("(kk p) t -> p kk t", p=128))
            for half in range(2):
                nc.sync.dma_start(out=eT[64 * half:64 * (half + 1), :],
                                  in_=demb[:, :])
                nc.sync.dma_start(out=erT[64 * half:64 * (half + 1), :],
                                  in_=dembr[:, :])
            nc.sync.dma_start(out=wvt[:, :, :],
                              in_=wv.rearrange("(kk p) t -> p kk t", p=128))
            nc.sync.dma_start(out=wdt[:, :, :],
                              in_=wd.rearrange("(kk p) t -> p kk t", p=128))

            def _proj_kq(g):
                for nh in range(2):
                    ps = pkd.tile([128, 512], FP32, tag="pkd")
                    for kk in range(6):
                        nc.tensor.matmul(ps[:, :],
                                         wkt[:, kk, 128 * g:128 * (g + 1)],
                                         xT[:, kk, 512 * nh:512 * (nh + 1)],
                                         start=(kk == 0), stop=(kk == 5))
                    nc.scalar.mul(kT[:, g, 512 * nh:512 * (nh + 1)],
                                  ps[:, :], SCALE)
                ps = pkd.tile([128, 512], FP32, tag="pkd")
                for kk in range(6):
                    nc.tensor.matmul(ps[:, :],
                                     wqt[:, kk, 128 * g:128 * (g + 1)],
                                     xqT[:, kk, :], start=(kk == 0),
                                     stop=(kk == 5))
                nc.scalar.mul(qT[:, g, :], ps[:, :], SCALE)

            def _emit_write_slot(g, J, kd_sb, qd_sb):
                for h01 in range(2):
                    hr = slice(64 * h01, 64 * (h01 + 1))
                    w0 = 896 - 128 * J
                    kdp = pkd.tile([128, KD_W], FP32, tag="pkd")
                    lhsT = kT[hr, g, 128 * J:128 * (J + 1)]
                    nc.tensor.matmul(kdp[:, 0:512], lhsT, eT[hr, w0:w0 + 512],
                                     start=True, stop=True)
                    nc.tensor.matmul(kdp[:, 512:KD_W], lhsT,
                                     eT[hr, w0 + 512:w0 + KD_W],
                                     start=True, stop=True)
                    nc.scalar.copy(kd_sb[:, J, h01, :], kdp[:, :])
                # one qd unit per slot, DVE-consumed from its own pool
                h01, I = J % 2, J // 2
                c0 = 384 - 128 * I
                Ip, half = I // 2, I % 2
                hr = slice(64 * h01, 64 * (h01 + 1))
                lhsT = qT[hr, g, 128 * I:128 * (I + 1)]
                qdpA = pqd.tile([128, 512], FP32, tag="pqd")
                nc.tensor.matmul(qdpA[:, :], lhsT, erT[hr, c0:c0 + 512],
                                 start=True, stop=True)
                nc.vector.tensor_copy(qd_sb[:, Ip, half, 0:512, h01],
                                      qdpA[:, :])
                qdpB = pqd.tile([128, 512], FP32, tag="pqd")
                nc.tensor.matmul(qdpB[:, 0:128], lhsT,
                                 erT[hr, c0 + 512:c0 + 640],
                                 start=True, stop=True)
                nc.tensor.matmul(qdpB[:, 128:512], lhsT,
                                 erT[hr, c0 + 640:c0 + 1024],
                                 start=True, stop=True)
                nc.vector.tensor_copy(
                    qd_sb[:, Ip, half, 512:1024, h01], qdpB[:, :])
                qdpC = pqd.tile([128, 128], FP32, tag="pqd")
                nc.tensor.matmul(qdpC[:, :], lhsT,
                                 erT[hr, c0 + 1024:c0 + 1152],
                                 start=True, stop=True)
                nc.vector.tensor_copy(
                    qd_sb[:, Ip, half, 1024:1152, h01], qdpC[:, :])
                # staged DRAM writes
                if J % 2 == 1:
                    Jp = J // 2
                    dst = kd_dram[g, 2 * Jp:2 * Jp + 2].transpose([2, 0, 1, 3])
                    nc.sync.dma_start(
                        out=dst, in_=kd_sb[:, 2 * Jp:2 * Jp + 2, :, :])
                if J == 7:
                    qd8 = qd_dram[g].bitcast(FP8)  # [512, 3072] fp8
                    dst = qd8[0:128, 0:2].unsqueeze(2).unsqueeze(3)
                    dst = _strided_view(
                        dst,
                        [(2 * QD_W, 128), (2 * (256 * QD_W - 256), 2),
                         (2 * (128 * QD_W - 128), 2), (1, 2304)],
                        2 * 384)
                    nc.sync.dma_start(
                        out=dst,
                        in_=qd_sb[:, :, :, :, :].rearrange(
                            "p a b u h -> p a b (u h)"))

            _proj_kq(0)
            for g in range(G):
                if g + 1 < G:
                    _proj_kq(g + 1)
                kd_sb = kdp_pool.tile([128, 8, 2, KD_W], FP8, tag="kd_sb")
                qd_sb = qdp_pool.tile([128, 2, 2, 1152, 2], FP8, tag="qd_sb")
                for J in range(8):
                    _emit_write_slot(g, J, kd_sb, qd_sb)
                # prefetch first shear reads late in W so phase S starts hot
                if g == 3:
                    _emit_reads(0)
                if g == 5:
                    _emit_reads(1)

        # ---------------- Phase S: shear reads + scores + PV --------------
        with tc.tile_pool(name="psS", bufs=4, space="PSUM") as pcs, \
             tc.tile_pool(name="psV", bufs=2, space="PSUM") as pv, \
             tc.tile_pool(name="psCTX", bufs=2, space="PSUM") as pctx, \
             tc.tile_pool(name="wkC", bufs=3) as wc, \
             tc.tile_pool(name="pP", bufs=6) as pp:

            def _emit_score_slot(g, J, ctxp):
                rq8 = rq_all[g][J][:, :].bitcast(FP8).rearrange(
                    "p (l two) -> p l two", two=2)
                for h01 in range(2):
                    h = 2 * g + h01
                    hr = slice(64 * h01, 64 * (h01 + 1))
                    relp = pcs.tile([128, LQ], FP32, tag="pcs")
                    rkv = rk_all[g][J // 2][:, 2 * (J % 2) + h01, :]
                    nc.vector.tensor_tensor(relp[:, :], rq8[:, :, h01],
                                            rkv, ADD)
                    nc.tensor.matmul(relp[:, :],
                                     kT[hr, g, 128 * J:128 * (J + 1)],
                                     qT[hr, g, :], start=False, stop=True)
                    p_sb = pp.tile([128, LQ], BF16, tag="p_sb")
                    nc.scalar.activation(p_sb[:, :], relp[:, :], EXP)
                    nc.tensor.matmul(ctxp[h01][:, :],
                                     vv[:, J, 65 * h:65 * h + 65],
                                     p_sb[:, :], start=(J == 0), stop=(J == 7))

            def _emit_normalize(g, ctxp):
                # normalize: ctxT_h = ctx' * (1/denom) broadcast over partitions
                for h01 in range(2):
                    hr = slice(64 * h01, 64 * (h01 + 1))
                    recip = wc.tile([1, LQ], BF16, tag="recip")
                    with nc.allow_low_precision(
                            reason="bf16 denom reciprocal; 0.4% scale error ok"):
                        nc.vector.reciprocal(recip[:, :], ctxp[h01][64:65, :])
                    bcp = pcs.tile([64, LQ], FP32, tag="pcs")
                    nc.tensor.matmul(bcp[:, :], ones_row[:, :], recip[:, :],
                                     start=True, stop=True)
                    bc_sb = wc.tile([64, LQ], FP32, tag="bc_sb")
                    nc.scalar.copy(bc_sb[:, :], bcp[:, :])
                    nc.vector.tensor_tensor(ctxT[hr, g, :], ctxp[h01][0:64, :],
                                            bc_sb[:, :], MULT)

            for g in range(G):
                if g + 2 < G:
                    _emit_reads(g + 2)
                ctxp = [pctx.tile([65, LQ], FP32, tag="ctxp",
                                  name=f"ctxp{i}") for i in range(2)]
                for r in range(8):
                    psv = pv.tile([128, 128], FP32, tag="pv")
                    for kk in range(6):
                        nc.tensor.matmul(psv[:, :],
                                         xT[:, kk, 128 * r:128 * (r + 1)],
                                         wvt[:, kk, 128 * g:128 * (g + 1)],
                                         start=(kk == 0), stop=(kk == 5))
                    vdst = vv[:, r, 130 * g:130 * g + 130].rearrange(
                        "p (h e) -> p h e", e=65)[:, :, 0:64]
                    vsrc = psv[:, :].rearrange("p (h e) -> p h e", e=64)
                    nc.scalar.copy(vdst, vsrc)
                for J in range(8):
                    _emit_score_slot(g, J, ctxp)
                _emit_normalize(g, ctxp)
                del rq_all[g], rk_all[g]

        # ---------------- Phase D: output projection ----------------
        with tc.tile_pool(name="psD", bufs=2, space="PSUM") as pd, \
             tc.tile_pool(name="oD", bufs=2) as od:
            for lc in range(4):
                ps = pd.tile([128, D], FP32, tag="outp")
                for kk in range(6):
                    for o, w in ((0, 512), (512, 256)):
                        nc.tensor.matmul(ps[:, o:o + w],
                                         ctxT[:, kk, 128 * lc:128 * (lc + 1)],
                                         wdt[:, kk, o:o + w],
                                         start=(kk == 0), stop=(kk == 5))
                o_sb = od.tile([128, D], FP32, tag="o_sb")
                nc.scalar.copy(o_sb[:, :], ps[:, :])
                nc.sync.dma_start(out=out[128 * lc:128 * (lc + 1), :], in_=o_sb[:, :])


_NC_CACHE = None


def _get_nc():
    global _NC_CACHE
    if _NC_CACHE is None:
        _NC_CACHE = build_nc()
    return _NC_CACHE


def make_in_maps(hidden_states, Wq, Wk, Wv, Wd, dist_emb):
    import ml_dtypes
    bf16 = ml_dtypes.bfloat16
    E = np.ascontiguousarray(np.asarray(dist_emb, np.float32))
    h32 = np.asarray(hidden_states, np.float32)
    Wqb = np.ascontiguousarray(np.asarray(Wq, np.float32).astype(bf16))
    Wkb = np.ascontiguousarray(np.asarray(Wk, np.float32).astype(bf16))
    Wvb = np.ascontiguousarray(np.asarray(Wv, np.float32).astype(bf16))
    Wdb = np.ascontiguousarray(np.asarray(Wd, np.float32).astype(bf16))
    in_maps = []
    for core in range(N_CORES):
        b, lh = core // 2, core % 2
        l0 = LQ * lh
        win = np.zeros((EW, HD), np.float32)
        n = min(EW, E.shape[0] - l0)
        win[:n] = E[l0:l0 + n]
        wins = win * np.float32(SCALE)
        in_maps.append({
            "hidden_q_T": np.ascontiguousarray(h32[b, l0:l0 + LQ].T.astype(bf16)),
            "hidden_kv_T": np.ascontiguousarray(h32[b].T.astype(bf16)),
            "Wq": Wqb, "Wk": Wkb, "Wv": Wvb, "Wd": Wdb,
            "demb_win_T": np.ascontiguousarray(wins.T.astype(bf16)),
            "demb_win_rev_T": np.ascontiguousarray(wins[::-1].T.astype(bf16)),
        })
    return in_maps


def run(inputs, trace=False):
    """Returns (full_output [4,1024,768], BassKernelResults)."""
    nc = _get_nc()
    in_maps = make_in_maps(inputs["hidden_states"], inputs["Wq"], inputs["Wk"],
                           inputs["Wv"], inputs["Wd"], inputs["dist_emb"])
    res = run_bass_kernel_spmd(nc, in_maps, list(range(N_CORES)), trace=trace)
    full = np.zeros((4, LK, D), np.float32)
    for core in range(N_CORES):
        b, lh = core // 2, core % 2
        full[b, LQ * lh:LQ * (lh + 1)] = res.results[core]["out"]
    return full, res


def kernel(**inputs):
    full, _ = run(inputs, trace=False)
    return full


if __name__ == "__main__":
    # quick self-build check
    nc = build_nc()
    print("built ok")
